# revision 7
# baseline (speedup 1.0000x reference)
"""AnomalyGNN (GCN encoder + linear decoder) on 8 TRN2 NeuronCores.

Strategy (dest-sharded, per the sharding hint):
 - Host folds D^{-1/2}[src] into x (diagonal scale), transposes + casts to
   bf16, and replicates it to all 8 cores.
 - Phase 1 (on HW, per core): s = (dinv*x) @ W_enc.T  for ALL nodes,
   written to DRAM as a bf16 row table (the GCN "message" table).
 - Phase 2: destinations are degree-sorted and dealt round-robin into
   8 core shards x 98 blocks x 128 slots. Each block's in-edges are
   gathered from the table with dma_gather (4 SWDGE queues, int16
   window-relative indices), then segment-summed into PSUM with
   one-hot indicator matmuls (indicator built on DVE from edge->dest
   metadata; the dest normalization dinv[dst] is folded into the
   indicator values). ReLU+bias on ACT gives z^T; PE transpose emits z;
   a second matmul + bias add emits the reconstruction.
 - Host unpermutes the slot-ordered outputs back to node order.
"""

import os
import sys

sys.path.insert(0, "/opt/trn_rl_repo")

import numpy as np
import ml_dtypes

import concourse.bass as bass
import concourse.bacc as bacc
import concourse.mybir as mybir
import concourse.tile as tile
from concourse.bass_utils import run_bass_kernel_spmd

BF16 = ml_dtypes.bfloat16
P = 128
NCORES = 8
SLAB = 512          # phase-1 node columns per matmul slab
MAXW = 25024        # max table-window rows (int16 indexing headroom)

LAST_EXEC_NS = None


def _roundup(a, b):
    return (a + b - 1) // b * b


def _wrap16(flat, cols):
    """[n] -> wrapped-16 layout [128, cols], replicated to 8 core groups."""
    a = np.zeros(cols * 16, dtype=np.int16)
    a[: len(flat)] = flat
    w = a.reshape(cols, 16).T  # [16, cols]
    return np.tile(w, (8, 1))


def _prep(x, edge_index, W_enc, b_enc, W_dec, b_dec):
    N, IN = x.shape
    HID = W_enc.shape[0]
    row = np.asarray(edge_index[0], dtype=np.int64)
    col = np.asarray(edge_index[1], dtype=np.int64)

    deg = (np.bincount(col, minlength=N) + 1).astype(np.float32)
    dinv = (1.0 / np.sqrt(deg)).astype(np.float32)

    # table windows (int16-relative indexing into the s table)
    W = max(1, -(-N // MAXW))
    TW = _roundup(-(-N // W), SLAB)
    assert TW <= 32512

    # destination sharding: degree-sorted deal-out
    order = np.argsort(-deg, kind="stable")  # node ids by desc degree
    core_of = np.empty(N, dtype=np.int32)
    slot_of = np.empty(N, dtype=np.int32)
    r = np.arange(N)
    core_of[order] = r % NCORES
    slot_of[order] = r // NCORES
    nsh = -(-N // NCORES)          # real slots per core
    slots = _roundup(nsh, P)       # padded slot grid
    NB = slots // P

    # edges + self-loops, keyed by (core, block, window)
    allrow = np.concatenate([row, np.arange(N, dtype=np.int64)])
    allcol = np.concatenate([col, np.arange(N, dtype=np.int64)])
    ec = core_of[allcol]
    es = slot_of[allcol]
    eJ = es // P
    ed = es % P
    ew = allrow // TW
    erel = (allrow % TW).astype(np.int16)
    key = ((ec * NB + eJ) * W + ew).astype(np.int64)
    ordr = np.argsort(key, kind="stable")
    key_s = key[ordr]
    erel_s = erel[ordr]
    ed_s = ed[ordr]
    ecol_s = allcol[ordr]
    counts = np.bincount(key_s, minlength=NCORES * NB * W).reshape(NCORES, NB, W)
    starts = np.zeros(NCORES * NB * W + 1, dtype=np.int64)
    np.cumsum(counts.reshape(-1), out=starts[1:])

    # static (SPMD-uniform) padded sizes
    NJW = np.zeros((NB, W), dtype=np.int64)          # padded idx count
    for Jb in range(NB):
        for w in range(W):
            NJW[Jb, w] = _roundup(int(counts[:, Jb, w].max()), P)
    GW = -(-NJW // P)                                 # chunks per window (ceil)
    GW[NJW == 0] = 0
    KB = GW.sum(axis=1)                               # chunks per block
    CWJ = NJW // 16                                   # idx cols per window

    # pack per-core flat buffers
    idx_cols = int(CWJ.sum())
    goff = np.zeros((NCORES, P, idx_cols), dtype=np.int16)
    kb_tot = int(KB.sum())
    crel = np.full((NCORES, P, kb_tot), -1.0, dtype=BF16)
    wval = np.zeros((NCORES, P, kb_tot), dtype=BF16)
    icol = np.zeros(NB * W + 1, dtype=np.int64)
    np.cumsum(CWJ.reshape(-1), out=icol[1:])
    koff = np.zeros(NB + 1, dtype=np.int64)
    np.cumsum(KB, out=koff[1:])

    for c in range(NCORES):
        for Jb in range(NB):
            kbase = koff[Jb]
            cbase = 0
            for w in range(W):
                njw = int(NJW[Jb, w])
                if njw == 0:
                    continue
                k = (c * NB + Jb) * W + w
                s0, s1 = starts[k], starts[k + 1]
                n = s1 - s0
                io = icol[Jb * W + w]
                goff[c, :, io : io + njw // 16] = _wrap16(erel_s[s0:s1], njw // 16)
                # edge i -> chunk kbase+cbase+i//128, partition i%128
                i = np.arange(n)
                ch = kbase + cbase + i // P
                pp = i % P
                crel[c, pp, ch] = ed_s[s0:s1].astype(np.float32)
                wval[c, pp, ch] = dinv[ecol_s[s0:s1]]
                cbase += GW[Jb, w]

    # replicated tensors
    xs = (x.astype(np.float32) * dinv[:, None]).T        # [IN, N]
    xt = np.zeros((IN, TW * W), dtype=BF16)
    xt[:, :N] = xs.astype(BF16)
    wenc = np.ascontiguousarray(W_enc.astype(np.float32).T).astype(BF16)  # [IN, HID]
    wdec = np.ascontiguousarray(W_dec.astype(np.float32).T).astype(BF16)  # [HID, IN]
    benc = np.asarray(b_enc, dtype=np.float32).reshape(HID, 1)
    bdec = np.tile(np.asarray(b_dec, dtype=np.float32)[None, :], (P, 1))  # [128, IN]
    iota = np.tile(np.arange(P, dtype=np.float32)[None, :], (P, 1)).astype(BF16)
    ident = np.eye(P, dtype=np.float32).astype(BF16)

    meta = dict(N=N, IN=IN, HID=HID, W=W, TW=TW, NB=NB, slots=slots, nsh=nsh,
                NJW=NJW, GW=GW, KB=KB, CWJ=CWJ, icol=icol, koff=koff,
                idx_cols=idx_cols, kb_tot=kb_tot)
    percore = dict(goff=goff, crel=crel, wval=wval)
    shared = dict(xt=xt, wenc=wenc, wdec=wdec, benc=benc, bdec=bdec,
                  iota=iota, ident=ident)
    unperm = (core_of, slot_of)
    return meta, percore, shared, unperm


def _build(meta):
    N, IN, HID = meta["N"], meta["IN"], meta["HID"]
    W, TW, NB = meta["W"], meta["TW"], meta["NB"]
    slots = meta["slots"]
    NJW, GW, KB, CWJ = meta["NJW"], meta["GW"], meta["KB"], meta["CWJ"]
    icol, koff = meta["icol"], meta["koff"]
    KIN = IN // P  # contraction chunks for encoder

    nc = bacc.Bacc("TRN2", target_bir_lowering=False, debug=False,
                   num_devices=NCORES, num_swdge_queues=4)
    f32, bf16, i16 = mybir.dt.float32, mybir.dt.bfloat16, mybir.dt.int16

    xt = nc.dram_tensor("xt", [IN, TW * W], bf16, kind="ExternalInput").ap()
    wenc = nc.dram_tensor("wenc", [IN, HID], bf16, kind="ExternalInput").ap()
    wdec = nc.dram_tensor("wdec", [HID, IN], bf16, kind="ExternalInput").ap()
    benc = nc.dram_tensor("benc", [HID, 1], f32, kind="ExternalInput").ap()
    bdec = nc.dram_tensor("bdec", [P, IN], f32, kind="ExternalInput").ap()
    iota = nc.dram_tensor("iota", [P, P], bf16, kind="ExternalInput").ap()
    ident = nc.dram_tensor("ident", [P, P], bf16, kind="ExternalInput").ap()
    goff = nc.dram_tensor("goff", [P, meta["idx_cols"]], i16, kind="ExternalInput").ap()
    crel = nc.dram_tensor("crel", [P, meta["kb_tot"]], bf16, kind="ExternalInput").ap()
    wval = nc.dram_tensor("wval", [P, meta["kb_tot"]], bf16, kind="ExternalInput").ap()
    dbg = os.environ.get("KERNEL_DEBUG", "0") == "1"
    z_out = nc.dram_tensor("z", [slots, HID], f32, kind="ExternalOutput").ap()
    sdbg = (nc.dram_tensor("sdbg", [W * TW, HID], mybir.dt.bfloat16,
                           kind="ExternalOutput").ap() if dbg else None)
    rec_out = nc.dram_tensor("recon", [slots, IN], f32, kind="ExternalOutput").ap()
    s_w = [nc.dram_tensor(f"s{w}", [TW, HID], bf16).ap() for w in range(W)]

    with tile.TileContext(nc) as tc:
        with tc.tile_pool(name="const", bufs=1) as cp, \
             tc.tile_pool(name="xtp", bufs=4) as xp, \
             tc.tile_pool(name="stage", bufs=3) as sp, \
             tc.tile_pool(name="gat", bufs=10) as gp, \
             tc.tile_pool(name="sel", bufs=3) as selp, \
             tc.tile_pool(name="meta", bufs=4) as mp, \
             tc.tile_pool(name="outp", bufs=3) as op, \
             tc.tile_pool(name="ps", bufs=2, space="PSUM") as pp, \
             tc.tile_pool(name="psz", bufs=2, space="PSUM") as pz:
            # PSUM budget: pp has tags ps1(+psT)[1 bank]x2 + psA[1 bank]x2,
            # pz has one shared tag [1 bank]x2 -> 6 banks total.

            wenc_tiles = []
            for k in range(KIN):
                t = cp.tile([P, HID], bf16, tag=f"wenc{k}")
                nc.sync.dma_start(out=t[:], in_=wenc[k * P:(k + 1) * P, :])
                wenc_tiles.append(t)
            wdec_t = cp.tile([HID, IN], bf16)
            nc.sync.dma_start(out=wdec_t[:], in_=wdec[:])
            benc_t = cp.tile([HID, 1], f32)
            nc.sync.dma_start(out=benc_t[:], in_=benc[:])
            bdec_t = cp.tile([P, IN], f32)
            nc.sync.dma_start(out=bdec_t[:], in_=bdec[:])
            iota_t = cp.tile([P, P], bf16)
            nc.sync.dma_start(out=iota_t[:], in_=iota[:])
            ident_t = cp.tile([P, P], bf16)
            nc.sync.dma_start(out=ident_t[:], in_=ident[:])

            # ---------------- Phase 1: s = xt_scaled @ W_enc.T (table build)
            for w in range(W):
                for off in range(0, TW, SLAB):
                    ns = min(SLAB, TW - off)
                    xts = []
                    for k in range(KIN):
                        t = xp.tile([P, ns], bf16, tag=f"xt{k}")
                        nc.sync.dma_start(
                            out=t[:], in_=xt[k * P:(k + 1) * P,
                                             w * TW + off: w * TW + off + ns])
                        xts.append(t)
                    ps1 = pp.tile([P, SLAB], f32, tag="ps1")
                    for k in range(KIN):
                        nc.tensor.matmul(ps1[:HID, :ns], lhsT=wenc_tiles[k][:],
                                         rhs=xts[k][:], start=(k == 0),
                                         stop=(k == KIN - 1))
                    st = sp.tile([P, SLAB], bf16, tag="st")
                    nc.scalar.activation(st[:HID, :ns], ps1[:HID, :ns],
                                         mybir.ActivationFunctionType.Copy)
                    psT = pp.tile([P, SLAB], bf16, tag="ps1")
                    nsub = -(-ns // P)
                    for t in range(nsub):
                        nc.tensor.transpose(psT[:, t * HID:t * HID + HID],
                                            st[:HID, t * P:t * P + P],
                                            ident_t[:])
                    sb2 = sp.tile([P, SLAB], bf16, tag="sb2")
                    nc.vector.tensor_copy(out=sb2[:, :nsub * HID],
                                          in_=psT[:, :nsub * HID])
                    nc.sync.dma_start(
                        out=s_w[w][off:off + ns, :].rearrange(
                            "(t p) f -> p t f", p=P),
                        in_=sb2[:].rearrange(
                            "p (t f) -> p t f", f=HID)[:, :nsub, :])

            tc.strict_bb_all_engine_barrier()

            if dbg:
                for w in range(W):
                    for off in range(0, TW, SLAB):
                        na = SLAB // P
                        dt_ = sp.tile([P, SLAB * HID // P], bf16, tag="dbg")
                        nc.sync.dma_start(
                            out=dt_[:].rearrange("p (a f) -> p a f", f=HID),
                            in_=s_w[w][off:off + SLAB, :].rearrange(
                                "(a p) f -> p a f", p=P))
                        nc.sync.dma_start(
                            out=sdbg[w * TW + off:w * TW + off + SLAB, :].rearrange(
                                "(a p) f -> p a f", p=P),
                            in_=dt_[:].rearrange("p (a f) -> p a f", f=HID))

            # ---------------- Phase 2: per dest block
            for Jb in range(NB):
                kb = int(KB[Jb])
                if kb == 0:
                    continue
                cw = int(CWJ[Jb].sum())
                it = mp.tile([P, max(cw, 1)], i16, tag="idx")
                nc.sync.dma_start(out=it[:, :cw],
                                  in_=goff[:, int(icol[Jb * W]):int(icol[Jb * W]) + cw])
                cr = mp.tile([P, kb], bf16, tag="cr")
                nc.sync.dma_start(out=cr[:], in_=crel[:, int(koff[Jb]):int(koff[Jb]) + kb])
                wv = mp.tile([P, kb], bf16, tag="wv")
                nc.sync.dma_start(out=wv[:], in_=wval[:, int(koff[Jb]):int(koff[Jb]) + kb])

                Gs = []
                for w in range(W):
                    gwn = int(GW[Jb, w])
                    if gwn == 0:
                        Gs.append(None)
                        continue
                    njw = int(NJW[Jb, w])
                    G = gp.tile([P, gwn * P], bf16, tag=f"G{w}")
                    io0 = int(icol[Jb * W + w] - icol[Jb * W])
                    nc.gpsimd.dma_gather(
                        out_ap=G[:].rearrange("p (k d) -> p k d", d=HID),
                        in_ap=s_w[w][:],
                        idxs_ap=it[:, io0: io0 + njw // 16],
                        num_idxs=njw,
                        num_idxs_reg=njw,
                        elem_size=HID,
                        single_packet=False,
                        queue_num=w % 4,
                    )
                    Gs.append(G)

                S = selp.tile([P, kb * P], bf16, tag="S")
                S3 = S[:].rearrange("p (k d) -> p k d", d=P)
                nc.vector.tensor_tensor(
                    out=S3, in0=cr[:].unsqueeze(2).to_broadcast([P, kb, P]),
                    in1=iota_t[:].unsqueeze(1).to_broadcast([P, kb, P]),
                    op=mybir.AluOpType.is_equal)
                nc.vector.tensor_tensor(
                    out=S3, in0=S3,
                    in1=wv[:].unsqueeze(2).to_broadcast([P, kb, P]),
                    op=mybir.AluOpType.mult)

                psA = pp.tile([P, P], f32, tag="psA")
                cglob = 0
                for w in range(W):
                    gwn = int(GW[Jb, w])
                    for lc in range(gwn):
                        nc.tensor.matmul(
                            psA[:HID, :], lhsT=Gs[w][:, lc * P:(lc + 1) * P],
                            rhs=S[:, cglob * P:(cglob + 1) * P],
                            start=(cglob == 0), stop=(cglob == kb - 1))
                        cglob += 1

                zt = sp.tile([HID, P], bf16, tag="zt")
                nc.scalar.activation(zt[:], psA[:HID, :],
                                     mybir.ActivationFunctionType.Relu,
                                     bias=benc_t[:, :1])

                psZ = pz.tile([P, HID], bf16, tag="pzz")
                nc.tensor.transpose(psZ[:, :HID], zt[:], ident_t[:])
                zf = op.tile([P, HID], f32, tag="zf")
                nc.scalar.activation(zf[:], psZ[:, :HID],
                                     mybir.ActivationFunctionType.Copy)
                nc.sync.dma_start(out=z_out[Jb * P:(Jb + 1) * P, :], in_=zf[:])

                psB = pz.tile([P, IN], f32, tag="pzz")
                nc.tensor.matmul(psB[:], lhsT=zt[:], rhs=wdec_t[:],
                                 start=True, stop=True)
                rec = op.tile([P, IN], f32, tag="rec")
                nc.vector.tensor_tensor(out=rec[:], in0=psB[:], in1=bdec_t[:],
                                        op=mybir.AluOpType.add)
                nc.sync.dma_start(out=rec_out[Jb * P:(Jb + 1) * P, :], in_=rec[:])

    nc.finalize()
    return nc


def kernel(x, edge_index, W_enc, b_enc, W_dec, b_dec):
    global LAST_EXEC_NS
    x = np.asarray(x)
    N, IN = x.shape
    HID = np.asarray(W_enc).shape[0]

    meta, percore, shared, (core_of, slot_of) = _prep(
        x, edge_index, W_enc, b_enc, W_dec, b_dec)
    nc = _build(meta)

    in_maps = []
    for c in range(NCORES):
        m = dict(shared)
        m["goff"] = percore["goff"][c]
        m["crel"] = percore["crel"][c]
        m["wval"] = percore["wval"][c]
        in_maps.append(m)

    trace = os.environ.get("KERNEL_TRACE", "0") == "1"
    res = run_bass_kernel_spmd(nc, in_maps, core_ids=list(range(NCORES)),
                               trace=trace)
    LAST_EXEC_NS = res.exec_time_ns

    z = np.empty((N, HID), dtype=np.float32)
    recon = np.empty((N, IN), dtype=np.float32)
    for c in range(NCORES):
        sel = core_of == c
        z[sel] = res.results[c]["z"][slot_of[sel]]
        recon[sel] = res.results[c]["recon"][slot_of[sel]]
    return z, recon
